# revision 1
# baseline (speedup 1.0000x reference)
"""Trainium2 Bass kernel for a 2-layer GCN (DGL GraphConv norm='both' with
EdgeWeightNorm) over a random graph: N=50000 nodes, E=800000 edges, F=128.

Strategy (8 NeuronCores, SPMD):
  - Nodes are partitioned contiguously across cores (6250 rows each); edges are
    owned by the dst-node owner (1D graph partitioning per the sharding hint).
  - All per-edge/per-node index manipulation (sorting, bucketing by dst tile,
    chunk padding, int16 gather-index layout, integer degree counts) happens on
    the host; all floating-point math runs on device.
  - Per layer, each core gathers xs[src] rows via the SWDGE dma_gather custom
    instruction (max 2048 indices per call; node table split into two <32768-row
    halves so indices fit int16), builds weighted one-hot matrices with a single
    fused tensor_scalar (is_equal then mult) per 128-edge chunk, and reduces
    chunks into PSUM with TensorE matmuls:
        aggT[f, d] += M_c[e, f]^T-contract  (lhsT=M_c, rhs=S_w)
    The dense layer is then h[d, fo] = matmul(lhsT=aggT, rhs=W) — no transposes.
  - Symmetric-norm scales fold per-node: src side into xs before the AllGather,
    dst side after the W matmul.  selu() is composed from Relu/Exp activations.
  - Features for the next layer are exchanged with an on-device AllGather
    (bounce through internal DRAM, Shared output), so the whole forward pass is
    a single NEFF launch.
"""

import math

import numpy as np

import concourse.bass as bass
import concourse.mybir as mybir
from concourse import bacc, tile
from concourse.bass_utils import run_bass_kernel_spmd

F = 128
P = 128
NCORES = 8
EPS = 1e-12
LAM = 1.0507009873554805
ALPHA = 1.6732632423543772

FP = mybir.dt.float32
I16 = mybir.dt.int16


def _ceil_div(a, b):
    return -(-a // b)


def _preprocess(src, dst, w, n_nodes):
    """Host-side index-only preprocessing. Returns (meta, per_core_inputs)."""
    NPC = n_nodes // NCORES          # nodes per core
    TPC = _ceil_div(NPC, P)          # dst tiles per core
    HALF = (n_nodes // 2 + 127) // 128 * 128  # split point for int16 indices
    assert HALF < 32768 and n_nodes - HALF < 32768

    E = src.shape[0]
    src = src.astype(np.int64)
    dst = dst.astype(np.int64)

    core = dst // NPC
    r = dst - core * NPC
    t_idx = r // P
    dst_rel = r - t_idx * P
    half = (src >= HALF).astype(np.int64)
    group = (core * TPC + t_idx) * 2 + half
    order = np.argsort(group, kind="stable")

    n_groups = NCORES * TPC * 2
    cnt = np.bincount(group, minlength=n_groups).reshape(NCORES, TPC, 2)
    C_th = _ceil_div(cnt, P).max(axis=0)          # [TPC, 2] chunks per (tile, half)
    assert C_th.max() <= 16, f"dma_gather limit exceeded: {C_th.max()} chunks"
    C_t = C_th.sum(axis=1)                        # [TPC]
    CT = int(C_t.sum())                           # total chunks per core
    chunk_off = np.zeros((TPC, 2), dtype=np.int64)
    flat = C_th.reshape(-1)
    chunk_off.reshape(-1)[1:] = np.cumsum(flat)[:-1]

    g_starts = np.zeros(n_groups + 1, dtype=np.int64)
    g_starts[1:] = np.cumsum(cnt.reshape(-1))

    # ---- per-node degree metadata (integer counts = CSR row sizes) ----
    in_deg = np.bincount(dst, minlength=n_nodes)
    out_deg = np.bincount(src, minlength=n_nodes)

    # node-padded weight layouts for on-device weighted-degree sums
    def padded_weights(key_nodes):
        o = np.argsort(key_nodes, kind="stable")
        kn = key_nodes[o]
        counts = np.bincount(key_nodes, minlength=n_nodes)
        starts = np.zeros(n_nodes, dtype=np.int64)
        starts[1:] = np.cumsum(counts)[:-1]
        slot = np.arange(E) - starts[kn]
        kcore = kn // NPC
        kr = kn - kcore * NPC
        kt = kr // P
        kp = kr - kt * P
        # md[t] = max slots needed in tile t over all cores
        md = np.zeros(TPC, dtype=np.int64)
        tile_of_edge = kcore * TPC + kt
        # max slot per (core,tile):
        mx = np.zeros(NCORES * TPC, dtype=np.int64)
        np.maximum.at(mx, tile_of_edge, slot + 1)
        md = mx.reshape(NCORES, TPC).max(axis=0)
        md = np.maximum(md, 1)
        moff = np.zeros(TPC, dtype=np.int64)
        moff[1:] = np.cumsum(md)[:-1]
        SW = int(md.sum())
        arrs = np.zeros((NCORES, P, SW), dtype=np.float32)
        arrs[kcore, kp, moff[kt] + slot] = w[o]
        return arrs, md, SW

    w_in_pad, md_in, SIN = padded_weights(dst)
    w_out_pad, md_out, SOUT = padded_weights(src)

    # integer degree columns [P, TPC] per core (clamped to >=1 as in reference)
    def deg_cols(deg):
        cols = np.ones((NCORES, P, TPC), dtype=np.float32)
        d = deg[: NCORES * NPC].reshape(NCORES, NPC)
        for k in range(NCORES):
            for t in range(TPC):
                lo = t * P
                hi = min(lo + P, NPC)
                cols[k, : hi - lo, t] = np.maximum(d[k, lo:hi], 1.0)
        return cols

    ideg_cols = deg_cols(in_deg)
    odeg_cols = deg_cols(out_deg)

    # ---- per-core chunked edge arrays ----
    GW = 8 * CT  # int16 index columns (NIDX/16 per chunk-call, 8 per chunk)
    dstrel_arr = np.full((NCORES, P, CT), 999.0, dtype=np.float32)
    w_arr = np.zeros((NCORES, P, CT), dtype=np.float32)
    gidx = np.zeros((NCORES, P, GW), dtype=np.int16)

    for k in range(NCORES):
        for t in range(TPC):
            for h in (0, 1):
                g = (k * TPC + t) * 2 + h
                s0, s1 = g_starts[g], g_starts[g + 1]
                n = s1 - s0
                C = int(C_th[t, h])
                if C == 0:
                    continue
                co = int(chunk_off[t, h])
                e_idx = order[s0:s1]
                i = np.arange(n)
                c = co + i // P
                p = i - (i // P) * P
                dstrel_arr[k, p, c] = dst_rel[e_idx]
                w_arr[k, p, c] = w[e_idx]
                iv = np.zeros(C * P, dtype=np.int16)
                iv[:n] = (src[e_idx] - h * HALF).astype(np.int16)
                w16 = iv.reshape(C * 8, 16).T            # [16, 8C]
                gidx[k, :, 8 * co : 8 * (co + C)] = np.tile(w16, (8, 1))

    meta = dict(
        NPC=NPC, TPC=TPC, HALF=HALF, CT=CT, GW=GW, SIN=SIN, SOUT=SOUT,
        C_th=C_th, C_t=C_t, chunk_off=chunk_off,
        md_in=md_in, md_out=md_out, n_nodes=n_nodes,
    )
    per_core = dict(
        dstrel=dstrel_arr, wchunk=w_arr, gidx=gidx,
        w_in_pad=w_in_pad, w_out_pad=w_out_pad,
        ideg=ideg_cols, odeg=odeg_cols,
    )
    return meta, per_core


def _build_program(meta):
    NPC, TPC, HALF = meta["NPC"], meta["TPC"], meta["HALF"]
    CT, GW = meta["CT"], meta["GW"]
    SIN, SOUT = meta["SIN"], meta["SOUT"]
    C_th, chunk_off = meta["C_th"], meta["chunk_off"]
    md_in, md_out = meta["md_in"], meta["md_out"]
    n_nodes = meta["n_nodes"]

    nc = bacc.Bacc("TRN2", target_bir_lowering=False, debug=False,
                   num_devices=NCORES, num_swdge_queues=4)

    x_local = nc.dram_tensor("x_local", [NPC, F], FP, kind="ExternalInput")
    gidx_in = nc.dram_tensor("gidx", [P, GW], I16, kind="ExternalInput")
    dstrel_in = nc.dram_tensor("dstrel", [P, CT], FP, kind="ExternalInput")
    wchunk_in = nc.dram_tensor("wchunk", [P, CT], FP, kind="ExternalInput")
    w_in_pad_in = nc.dram_tensor("w_in_pad", [P, SIN], FP, kind="ExternalInput")
    w_out_pad_in = nc.dram_tensor("w_out_pad", [P, SOUT], FP, kind="ExternalInput")
    ideg_in = nc.dram_tensor("ideg", [P, TPC], FP, kind="ExternalInput")
    odeg_in = nc.dram_tensor("odeg", [P, TPC], FP, kind="ExternalInput")
    W1_in = nc.dram_tensor("W1", [F, F], FP, kind="ExternalInput")
    W2_in = nc.dram_tensor("W2", [F, F], FP, kind="ExternalInput")
    b1b_in = nc.dram_tensor("b1b", [P, F], FP, kind="ExternalInput")
    b2b_in = nc.dram_tensor("b2b", [P, F], FP, kind="ExternalInput")
    iota_in = nc.dram_tensor("iota", [P, P], FP, kind="ExternalInput")
    out_ext = nc.dram_tensor("out", [NPC, F], FP, kind="ExternalOutput")

    last_rows = NPC - (TPC - 1) * P  # valid rows in the final tile

    with tile.TileContext(nc) as tc:
        with tc.tile_pool(name="const", bufs=1) as const, \
             tc.tile_pool(name="dram", bufs=1, space="DRAM") as dram, \
             tc.tile_pool(name="cols", bufs=1) as cols, \
             tc.tile_pool(name="gpool", bufs=6) as gpool, \
             tc.tile_pool(name="spool", bufs=4) as spool, \
             tc.tile_pool(name="work", bufs=3) as work, \
             tc.tile_pool(name="pagg", bufs=2, space="PSUM") as pagg, \
             tc.tile_pool(name="ph", bufs=2, space="PSUM") as ph:

            # ---------- constants ----------
            iota_t = const.tile([P, P], FP)
            nc.sync.dma_start(out=iota_t[:], in_=iota_in[:])
            W1_t = const.tile([F, F], FP)
            nc.sync.dma_start(out=W1_t[:], in_=W1_in[:])
            W2_t = const.tile([F, F], FP)
            nc.sync.dma_start(out=W2_t[:], in_=W2_in[:])
            b1b_t = const.tile([P, F], FP)
            nc.sync.dma_start(out=b1b_t[:], in_=b1b_in[:])
            b2b_t = const.tile([P, F], FP)
            nc.sync.dma_start(out=b2b_t[:], in_=b2b_in[:])
            gidx_t = const.tile([P, GW], I16)
            nc.sync.dma_start(out=gidx_t[:], in_=gidx_in[:])
            dstrel_t = const.tile([P, CT], FP)
            nc.sync.dma_start(out=dstrel_t[:], in_=dstrel_in[:])
            wchunk_t = const.tile([P, CT], FP)
            nc.sync.dma_start(out=wchunk_t[:], in_=wchunk_in[:])

            # ---------- per-node scales ----------
            wip_t = cols.tile([P, SIN], FP)
            nc.sync.dma_start(out=wip_t[:], in_=w_in_pad_in[:])
            wop_t = cols.tile([P, SOUT], FP)
            nc.sync.dma_start(out=wop_t[:], in_=w_out_pad_in[:])
            ideg_t = cols.tile([P, TPC], FP)
            nc.sync.dma_start(out=ideg_t[:], in_=ideg_in[:])
            odeg_t = cols.tile([P, TPC], FP)
            nc.sync.dma_start(out=odeg_t[:], in_=odeg_in[:])

            wdeg_in = cols.tile([P, TPC], FP)
            wdeg_out = cols.tile([P, TPC], FP)
            o = 0
            for t in range(TPC):
                nc.vector.reduce_sum(out=wdeg_in[:, t : t + 1],
                                     in_=wip_t[:, o : o + int(md_in[t])],
                                     axis=mybir.AxisListType.X)
                o += int(md_in[t])
            o = 0
            for t in range(TPC):
                nc.vector.reduce_sum(out=wdeg_out[:, t : t + 1],
                                     in_=wop_t[:, o : o + int(md_out[t])],
                                     axis=mybir.AxisListType.X)
                o += int(md_out[t])

            def rsqrt_cols(dst_tile, src_ap, clamp_eps):
                tmp = cols.tile([P, TPC], FP, name=f"tmp_{dst_tile.name}")
                if clamp_eps:
                    nc.vector.tensor_scalar(out=tmp[:], in0=src_ap, scalar1=EPS,
                                            scalar2=None, op0=mybir.AluOpType.max)
                    nc.scalar.activation(tmp[:], tmp[:],
                                         mybir.ActivationFunctionType.Sqrt)
                else:
                    nc.scalar.activation(tmp[:], src_ap,
                                         mybir.ActivationFunctionType.Sqrt)
                nc.vector.reciprocal(dst_tile[:], tmp[:])

            inv_in = cols.tile([P, TPC], FP)
            rsqrt_cols(inv_in, wdeg_in[:], True)
            r_in = cols.tile([P, TPC], FP)
            rsqrt_cols(r_in, ideg_t[:], False)
            s_in = cols.tile([P, TPC], FP)
            nc.vector.tensor_tensor(out=s_in[:], in0=inv_in[:], in1=r_in[:],
                                    op=mybir.AluOpType.mult)

            inv_out = cols.tile([P, TPC], FP)
            rsqrt_cols(inv_out, wdeg_out[:], True)
            r_out = cols.tile([P, TPC], FP)
            rsqrt_cols(r_out, odeg_t[:], False)
            s_out = cols.tile([P, TPC], FP)
            nc.vector.tensor_tensor(out=s_out[:], in0=inv_out[:], in1=r_out[:],
                                    op=mybir.AluOpType.mult)

            s2 = cols.tile([P, TPC], FP)   # lambda * s_out
            nc.vector.tensor_scalar(out=s2[:], in0=s_out[:], scalar1=LAM,
                                    scalar2=None, op0=mybir.AluOpType.mult)
            s3 = cols.tile([P, TPC], FP)   # lambda*alpha * s_out
            nc.vector.tensor_scalar(out=s3[:], in0=s_out[:], scalar1=LAM * ALPHA,
                                    scalar2=None, op0=mybir.AluOpType.mult)
            s3n = cols.tile([P, TPC], FP)  # -lambda*alpha * s_out
            nc.vector.tensor_scalar(out=s3n[:], in0=s_out[:], scalar1=-LAM * ALPHA,
                                    scalar2=None, op0=mybir.AluOpType.mult)

            # ---------- xs1 = x * s_out, then AllGather ----------
            ag1_in = dram.tile([NPC, F], FP)
            xs_full = dram.tile([n_nodes, F], FP, addr_space="Shared")
            ag2_in = dram.tile([NPC, F], FP)
            xs2_full = dram.tile([n_nodes, F], FP, addr_space="Shared")

            for t in range(TPC):
                rows = last_rows if t == TPC - 1 else P
                xt = work.tile([P, F], FP, tag="xs_stage")
                nc.sync.dma_start(out=xt[:rows, :],
                                  in_=x_local[t * P : t * P + rows, :])
                nc.vector.tensor_scalar(out=xt[:rows, :], in0=xt[:rows, :],
                                        scalar1=s_out[:rows, t : t + 1],
                                        scalar2=None, op0=mybir.AluOpType.mult)
                nc.sync.dma_start(out=ag1_in[t * P : t * P + rows, :],
                                  in_=xt[:rows, :])

            nc.gpsimd.collective_compute(
                "AllGather", mybir.AluOpType.bypass,
                replica_groups=[list(range(NCORES))],
                ins=[ag1_in[:]], outs=[xs_full[:]],
            )

            # ---------- the two graph-conv layers ----------
            qrr = [0]

            def layer(xsrc, W_t, bb_t, is_last):
                for t in range(TPC):
                    rows = last_rows if t == TPC - 1 else P
                    C_t_total = int(C_th[t, 0] + C_th[t, 1])
                    # gather the tile's edges (one call per node-half)
                    gtiles = {}
                    for h in (0, 1):
                        C = int(C_th[t, h])
                        if C == 0:
                            continue
                        co = int(chunk_off[t, h])
                        g = gpool.tile([P, C, F], FP, tag=f"g{h}", name=f"g_{t}_{h}")
                        nidx = C * P
                        nc.gpsimd.dma_gather(
                            g[:],
                            xsrc[h * HALF : h * HALF + (n_nodes - HALF if h else HALF), :],
                            gidx_t[:, 8 * co : 8 * (co + C)],
                            nidx, nidx, F,
                            single_packet=False,
                            queue_num=qrr[0] % 4,
                        )
                        qrr[0] += 1
                        gtiles[h] = g

                    agg = pagg.tile([P, P], FP, tag="agg")
                    ci = 0
                    for h in (0, 1):
                        C = int(C_th[t, h])
                        if C == 0:
                            continue
                        co = int(chunk_off[t, h])
                        g = gtiles[h]
                        for c in range(C):
                            gc = co + c
                            sw = spool.tile([P, P], FP, tag="sw", name=f"sw_{t}_{h}_{c}")
                            nc.vector.tensor_scalar(
                                out=sw[:], in0=iota_t[:],
                                scalar1=dstrel_t[:, gc : gc + 1],
                                scalar2=wchunk_t[:, gc : gc + 1],
                                op0=mybir.AluOpType.is_equal,
                                op1=mybir.AluOpType.mult,
                            )
                            nc.tensor.matmul(
                                out=agg[:], lhsT=g[:, c, :], rhs=sw[:],
                                start=(ci == 0), stop=(ci == C_t_total - 1),
                            )
                            ci += 1

                    aggT_sb = work.tile([P, P], FP, tag="aggT")
                    nc.scalar.copy(aggT_sb[:], agg[:])
                    hp = ph.tile([P, F], FP, tag="hp")
                    nc.tensor.matmul(out=hp[:], lhsT=aggT_sb[:], rhs=W_t[:],
                                     start=True, stop=True)

                    v = work.tile([P, F], FP, tag="v")
                    nc.vector.tensor_scalar(out=v[:], in0=hp[:],
                                            scalar1=s_in[:, t : t + 1],
                                            scalar2=None,
                                            op0=mybir.AluOpType.mult)
                    nc.vector.tensor_tensor(out=v[:], in0=v[:], in1=bb_t[:],
                                            op=mybir.AluOpType.add)
                    u = work.tile([P, F], FP, tag="u")
                    nc.scalar.activation(u[:], v[:],
                                         mybir.ActivationFunctionType.Relu,
                                         scale=-1.0)
                    ex = work.tile([P, F], FP, tag="ex")
                    nc.scalar.activation(ex[:], u[:],
                                         mybir.ActivationFunctionType.Exp,
                                         scale=-1.0)
                    r = work.tile([P, F], FP, tag="r")
                    tt = work.tile([P, F], FP, tag="tt")
                    ot = work.tile([P, F], FP, tag="ot")
                    if is_last:
                        nc.scalar.activation(r[:], v[:],
                                             mybir.ActivationFunctionType.Relu,
                                             scale=LAM)
                        nc.vector.tensor_scalar(out=tt[:], in0=ex[:],
                                                scalar1=LAM * ALPHA,
                                                scalar2=-LAM * ALPHA,
                                                op0=mybir.AluOpType.mult,
                                                op1=mybir.AluOpType.add)
                        nc.vector.tensor_tensor(out=ot[:], in0=r[:], in1=tt[:],
                                                op=mybir.AluOpType.add)
                        nc.sync.dma_start(
                            out=out_ext[t * P : t * P + rows, :],
                            in_=ot[:rows, :])
                    else:
                        nc.scalar.activation(r[:], v[:],
                                             mybir.ActivationFunctionType.Relu,
                                             scale=s2[:, t : t + 1])
                        nc.vector.tensor_scalar(out=tt[:], in0=ex[:],
                                                scalar1=s3[:, t : t + 1],
                                                scalar2=s3n[:, t : t + 1],
                                                op0=mybir.AluOpType.mult,
                                                op1=mybir.AluOpType.add)
                        nc.vector.tensor_tensor(out=ot[:], in0=r[:], in1=tt[:],
                                                op=mybir.AluOpType.add)
                        nc.sync.dma_start(
                            out=ag2_in[t * P : t * P + rows, :],
                            in_=ot[:rows, :])

            layer(xs_full, W1_t, b1b_t, is_last=False)
            nc.gpsimd.collective_compute(
                "AllGather", mybir.AluOpType.bypass,
                replica_groups=[list(range(NCORES))],
                ins=[ag2_in[:]], outs=[xs2_full[:]],
            )
            layer(xs2_full, W2_t, b2b_t, is_last=True)

    nc.compile()
    return nc


_CACHE = {}


def _get_program(meta_key, meta):
    if meta_key not in _CACHE:
        _CACHE[meta_key] = _build_program(meta)
    return _CACHE[meta_key]


def kernel(x, src, dst, edge_w, W1, b1, W2, b2):
    x = np.asarray(x, dtype=np.float32)
    src_np = np.asarray(src)
    dst_np = np.asarray(dst)
    w_np = np.asarray(edge_w, dtype=np.float32)
    W1 = np.asarray(W1, dtype=np.float32)
    b1 = np.asarray(b1, dtype=np.float32)
    W2 = np.asarray(W2, dtype=np.float32)
    b2 = np.asarray(b2, dtype=np.float32)

    n_nodes = x.shape[0]
    meta, per_core = _preprocess(src_np, dst_np, w_np, n_nodes)
    NPC = meta["NPC"]

    meta_key = (
        n_nodes, src_np.shape[0],
        meta["CT"], meta["SIN"], meta["SOUT"],
        tuple(meta["C_th"].reshape(-1).tolist()),
        tuple(meta["md_in"].tolist()), tuple(meta["md_out"].tolist()),
    )
    nc = _get_program(meta_key, meta)

    iota = np.broadcast_to(np.arange(P, dtype=np.float32), (P, P)).copy()
    b1b = np.broadcast_to(b1, (P, F)).copy()
    b2b = np.broadcast_to(b2, (P, F)).copy()

    in_maps = []
    for k in range(NCORES):
        in_maps.append({
            "x_local": x[k * NPC : (k + 1) * NPC],
            "gidx": per_core["gidx"][k],
            "dstrel": per_core["dstrel"][k],
            "wchunk": per_core["wchunk"][k],
            "w_in_pad": per_core["w_in_pad"][k],
            "w_out_pad": per_core["w_out_pad"][k],
            "ideg": per_core["ideg"][k],
            "odeg": per_core["odeg"][k],
            "W1": W1, "W2": W2, "b1b": b1b, "b2b": b2b,
            "iota": iota,
        })

    res = run_bass_kernel_spmd(nc, in_maps, core_ids=list(range(NCORES)))
    out = np.concatenate([res.results[k]["out"] for k in range(NCORES)], axis=0)
    return out.astype(np.float32)


# revision 3
# speedup vs baseline: 1.4499x; 1.4499x over previous
"""Trainium2 Bass kernel for a 2-layer GCN (DGL GraphConv norm='both' with
EdgeWeightNorm) over a random graph: N=50000 nodes, E=800000 edges, F=128.

Strategy (8 NeuronCores, SPMD):
  - Nodes are partitioned contiguously across cores (6250 rows each); edges are
    owned by the dst-node owner (1D graph partitioning per the sharding hint).
  - All per-edge/per-node index manipulation (sorting, bucketing by dst tile,
    chunk padding, int16 gather-index layout, integer degree counts) happens on
    the host; all floating-point math runs on device.
  - Per layer, each core gathers xs[src] rows via the SWDGE dma_gather custom
    instruction (max 2048 indices per call; node table split into two <32768-row
    halves so indices fit int16), builds weighted one-hot matrices with a single
    fused tensor_scalar (is_equal then mult) per 128-edge chunk, and reduces
    chunks into PSUM with TensorE matmuls:
        aggT[f, d] += M_c[e, f]^T-contract  (lhsT=M_c, rhs=S_w)
    The dense layer is then h[d, fo] = matmul(lhsT=aggT, rhs=W) — no transposes.
  - Symmetric-norm scales fold per-node: src side into xs before the AllGather,
    dst side after the W matmul.  selu() is composed from Relu/Exp activations.
  - Features for the next layer are exchanged with an on-device AllGather
    (bounce through internal DRAM, Shared output), so the whole forward pass is
    a single NEFF launch.
"""

import math

import ml_dtypes
import numpy as np

import concourse.bass as bass
import concourse.mybir as mybir
from concourse import bacc, tile
from concourse.bass_utils import run_bass_kernel_spmd

F = 128
P = 128
NCORES = 8
EPS = 1e-12
LAM = 1.0507009873554805
ALPHA = 1.6732632423543772

FP = mybir.dt.float32
BF = mybir.dt.bfloat16
I16 = mybir.dt.int16


def _ceil_div(a, b):
    return -(-a // b)


def _preprocess(src, dst, w, n_nodes):
    """Host-side index-only preprocessing. Returns (meta, per_core_inputs)."""
    NPC = n_nodes // NCORES          # nodes per core
    TPC = _ceil_div(NPC, P)          # dst tiles per core
    HALF = (n_nodes // 2 + 127) // 128 * 128  # split point for int16 indices
    assert HALF < 32768 and n_nodes - HALF < 32768

    E = src.shape[0]
    src = src.astype(np.int64)
    dst = dst.astype(np.int64)

    core = dst // NPC
    r = dst - core * NPC
    t_idx = r // P
    dst_rel = r - t_idx * P
    half = (src >= HALF).astype(np.int64)
    group = (core * TPC + t_idx) * 2 + half
    order = np.argsort(group, kind="stable")

    n_groups = NCORES * TPC * 2
    cnt = np.bincount(group, minlength=n_groups).reshape(NCORES, TPC, 2)
    C_th = _ceil_div(cnt, P).max(axis=0)          # [TPC, 2] chunks per (tile, half)
    assert C_th.max() <= 16, f"dma_gather limit exceeded: {C_th.max()} chunks"
    C_t = C_th.sum(axis=1)                        # [TPC]
    CT = int(C_t.sum())                           # total chunks per core
    chunk_off = np.zeros((TPC, 2), dtype=np.int64)
    flat = C_th.reshape(-1)
    chunk_off.reshape(-1)[1:] = np.cumsum(flat)[:-1]

    g_starts = np.zeros(n_groups + 1, dtype=np.int64)
    g_starts[1:] = np.cumsum(cnt.reshape(-1))

    # ---- per-node degree metadata (integer counts = CSR row sizes) ----
    in_deg = np.bincount(dst, minlength=n_nodes)
    out_deg = np.bincount(src, minlength=n_nodes)

    # node-padded weight layouts for on-device weighted-degree sums
    def padded_weights(key_nodes):
        o = np.argsort(key_nodes, kind="stable")
        kn = key_nodes[o]
        counts = np.bincount(key_nodes, minlength=n_nodes)
        starts = np.zeros(n_nodes, dtype=np.int64)
        starts[1:] = np.cumsum(counts)[:-1]
        slot = np.arange(E) - starts[kn]
        kcore = kn // NPC
        kr = kn - kcore * NPC
        kt = kr // P
        kp = kr - kt * P
        # md[t] = max slots needed in tile t over all cores
        md = np.zeros(TPC, dtype=np.int64)
        tile_of_edge = kcore * TPC + kt
        # max slot per (core,tile):
        mx = np.zeros(NCORES * TPC, dtype=np.int64)
        np.maximum.at(mx, tile_of_edge, slot + 1)
        md = mx.reshape(NCORES, TPC).max(axis=0)
        md = np.maximum(md, 1)
        moff = np.zeros(TPC, dtype=np.int64)
        moff[1:] = np.cumsum(md)[:-1]
        SW = int(md.sum())
        arrs = np.zeros((NCORES, P, SW), dtype=np.float32)
        arrs[kcore, kp, moff[kt] + slot] = w[o]
        return arrs, md, SW

    w_in_pad, md_in, SIN = padded_weights(dst)
    w_out_pad, md_out, SOUT = padded_weights(src)

    # integer degree columns [P, TPC] per core (clamped to >=1 as in reference)
    def deg_cols(deg):
        cols = np.ones((NCORES, P, TPC), dtype=np.float32)
        d = deg[: NCORES * NPC].reshape(NCORES, NPC)
        for k in range(NCORES):
            for t in range(TPC):
                lo = t * P
                hi = min(lo + P, NPC)
                cols[k, : hi - lo, t] = np.maximum(d[k, lo:hi], 1.0)
        return cols

    ideg_cols = deg_cols(in_deg)
    odeg_cols = deg_cols(out_deg)

    # ---- per-core chunked edge arrays ----
    GW = 8 * CT  # int16 index columns (NIDX/16 per chunk-call, 8 per chunk)
    dstrel_arr = np.full((NCORES, P, CT), 999.0, dtype=np.float32)
    w_arr = np.zeros((NCORES, P, CT), dtype=np.float32)
    gidx = np.zeros((NCORES, P, GW), dtype=np.int16)

    for k in range(NCORES):
        for t in range(TPC):
            for h in (0, 1):
                g = (k * TPC + t) * 2 + h
                s0, s1 = g_starts[g], g_starts[g + 1]
                n = s1 - s0
                C = int(C_th[t, h])
                if C == 0:
                    continue
                co = int(chunk_off[t, h])
                e_idx = order[s0:s1]
                i = np.arange(n)
                c = co + i // P
                p = i - (i // P) * P
                dstrel_arr[k, p, c] = dst_rel[e_idx]
                w_arr[k, p, c] = w[e_idx]
                iv = np.zeros(C * P, dtype=np.int16)
                iv[:n] = (src[e_idx] - h * HALF).astype(np.int16)
                w16 = iv.reshape(C * 8, 16).T            # [16, 8C]
                gidx[k, :, 8 * co : 8 * (co + C)] = np.tile(w16, (8, 1))

    meta = dict(
        NPC=NPC, TPC=TPC, HALF=HALF, CT=CT, GW=GW, SIN=SIN, SOUT=SOUT,
        C_th=C_th, C_t=C_t, chunk_off=chunk_off,
        md_in=md_in, md_out=md_out, n_nodes=n_nodes,
    )
    per_core = dict(
        dstrel=dstrel_arr, wchunk=w_arr, gidx=gidx,
        w_in_pad=w_in_pad, w_out_pad=w_out_pad,
        ideg=ideg_cols, odeg=odeg_cols,
    )
    return meta, per_core


def _build_program(meta):
    NPC, TPC, HALF = meta["NPC"], meta["TPC"], meta["HALF"]
    CT, GW = meta["CT"], meta["GW"]
    SIN, SOUT = meta["SIN"], meta["SOUT"]
    C_th, chunk_off = meta["C_th"], meta["chunk_off"]
    md_in, md_out = meta["md_in"], meta["md_out"]
    n_nodes = meta["n_nodes"]

    nc = bacc.Bacc("TRN2", target_bir_lowering=False, debug=False,
                   num_devices=NCORES, num_swdge_queues=4)

    x_local = nc.dram_tensor("x_local", [NPC, F], FP, kind="ExternalInput")
    gidx_in = nc.dram_tensor("gidx", [P, GW], I16, kind="ExternalInput")
    dstrel_in = nc.dram_tensor("dstrel", [P, CT], FP, kind="ExternalInput")
    wchunk_in = nc.dram_tensor("wchunk", [P, CT], FP, kind="ExternalInput")
    w_in_pad_in = nc.dram_tensor("w_in_pad", [P, SIN], FP, kind="ExternalInput")
    w_out_pad_in = nc.dram_tensor("w_out_pad", [P, SOUT], FP, kind="ExternalInput")
    ideg_in = nc.dram_tensor("ideg", [P, TPC], FP, kind="ExternalInput")
    odeg_in = nc.dram_tensor("odeg", [P, TPC], FP, kind="ExternalInput")
    W1_in = nc.dram_tensor("W1", [F, F], FP, kind="ExternalInput")
    W2_in = nc.dram_tensor("W2", [F, F], FP, kind="ExternalInput")
    b1b_in = nc.dram_tensor("b1b", [P, F], FP, kind="ExternalInput")
    b2b_in = nc.dram_tensor("b2b", [P, F], FP, kind="ExternalInput")
    iota_in = nc.dram_tensor("iota", [P, P], BF, kind="ExternalInput")
    out_ext = nc.dram_tensor("out", [NPC, F], FP, kind="ExternalOutput")

    last_rows = NPC - (TPC - 1) * P  # valid rows in the final tile

    with tile.TileContext(nc) as tc:
        with tc.tile_pool(name="const", bufs=1) as const, \
             tc.tile_pool(name="dram", bufs=1, space="DRAM") as dram, \
             tc.tile_pool(name="cols", bufs=1) as cols, \
             tc.tile_pool(name="gpool", bufs=8) as gpool, \
             tc.tile_pool(name="spool", bufs=6) as spool, \
             tc.tile_pool(name="work", bufs=3) as work, \
             tc.tile_pool(name="pagg", bufs=2, space="PSUM") as pagg, \
             tc.tile_pool(name="ph", bufs=2, space="PSUM") as ph:

            # ---------- constants ----------
            iota_t = const.tile([P, P], BF)
            nc.sync.dma_start(out=iota_t[:], in_=iota_in[:])
            W1_t = const.tile([F, F], FP)
            nc.sync.dma_start(out=W1_t[:], in_=W1_in[:])
            W2_t = const.tile([F, F], FP)
            nc.sync.dma_start(out=W2_t[:], in_=W2_in[:])
            b1b_t = const.tile([P, F], FP)
            nc.sync.dma_start(out=b1b_t[:], in_=b1b_in[:])
            b2b_t = const.tile([P, F], FP)
            nc.sync.dma_start(out=b2b_t[:], in_=b2b_in[:])
            gidx_t = const.tile([P, GW], I16)
            nc.sync.dma_start(out=gidx_t[:], in_=gidx_in[:])
            dstrel_t = const.tile([P, CT], FP)
            nc.sync.dma_start(out=dstrel_t[:], in_=dstrel_in[:])
            wchunk_t = const.tile([P, CT], FP)
            nc.sync.dma_start(out=wchunk_t[:], in_=wchunk_in[:])

            # ---------- per-node scales ----------
            wip_t = cols.tile([P, SIN], FP)
            nc.sync.dma_start(out=wip_t[:], in_=w_in_pad_in[:])
            wop_t = cols.tile([P, SOUT], FP)
            nc.sync.dma_start(out=wop_t[:], in_=w_out_pad_in[:])
            ideg_t = cols.tile([P, TPC], FP)
            nc.sync.dma_start(out=ideg_t[:], in_=ideg_in[:])
            odeg_t = cols.tile([P, TPC], FP)
            nc.sync.dma_start(out=odeg_t[:], in_=odeg_in[:])

            wdeg_in = cols.tile([P, TPC], FP)
            wdeg_out = cols.tile([P, TPC], FP)
            o = 0
            for t in range(TPC):
                nc.vector.reduce_sum(out=wdeg_in[:, t : t + 1],
                                     in_=wip_t[:, o : o + int(md_in[t])],
                                     axis=mybir.AxisListType.X)
                o += int(md_in[t])
            o = 0
            for t in range(TPC):
                nc.vector.reduce_sum(out=wdeg_out[:, t : t + 1],
                                     in_=wop_t[:, o : o + int(md_out[t])],
                                     axis=mybir.AxisListType.X)
                o += int(md_out[t])

            def rsqrt_cols(dst_tile, src_ap, clamp_eps):
                tmp = cols.tile([P, TPC], FP, name=f"tmp_{dst_tile.name}")
                if clamp_eps:
                    nc.vector.tensor_scalar(out=tmp[:], in0=src_ap, scalar1=EPS,
                                            scalar2=None, op0=mybir.AluOpType.max)
                    nc.scalar.activation(tmp[:], tmp[:],
                                         mybir.ActivationFunctionType.Sqrt)
                else:
                    nc.scalar.activation(tmp[:], src_ap,
                                         mybir.ActivationFunctionType.Sqrt)
                nc.vector.reciprocal(dst_tile[:], tmp[:])

            inv_in = cols.tile([P, TPC], FP)
            rsqrt_cols(inv_in, wdeg_in[:], True)
            r_in = cols.tile([P, TPC], FP)
            rsqrt_cols(r_in, ideg_t[:], False)
            s_in = cols.tile([P, TPC], FP)
            nc.vector.tensor_tensor(out=s_in[:], in0=inv_in[:], in1=r_in[:],
                                    op=mybir.AluOpType.mult)

            inv_out = cols.tile([P, TPC], FP)
            rsqrt_cols(inv_out, wdeg_out[:], True)
            r_out = cols.tile([P, TPC], FP)
            rsqrt_cols(r_out, odeg_t[:], False)
            s_out = cols.tile([P, TPC], FP)
            nc.vector.tensor_tensor(out=s_out[:], in0=inv_out[:], in1=r_out[:],
                                    op=mybir.AluOpType.mult)

            s2 = cols.tile([P, TPC], FP)   # lambda * s_out
            nc.vector.tensor_scalar(out=s2[:], in0=s_out[:], scalar1=LAM,
                                    scalar2=None, op0=mybir.AluOpType.mult)
            s3 = cols.tile([P, TPC], FP)   # lambda*alpha * s_out
            nc.vector.tensor_scalar(out=s3[:], in0=s_out[:], scalar1=LAM * ALPHA,
                                    scalar2=None, op0=mybir.AluOpType.mult)
            s3n = cols.tile([P, TPC], FP)  # -lambda*alpha * s_out
            nc.vector.tensor_scalar(out=s3n[:], in0=s_out[:], scalar1=-LAM * ALPHA,
                                    scalar2=None, op0=mybir.AluOpType.mult)

            # ---------- xs1 = x * s_out, then AllGather ----------
            ag1_in = dram.tile([NPC, F], BF)
            xs_full = dram.tile([n_nodes, F], BF, addr_space="Shared")
            ag2_in = dram.tile([NPC, F], BF)
            xs2_full = dram.tile([n_nodes, F], BF, addr_space="Shared")

            for t in range(TPC):
                rows = last_rows if t == TPC - 1 else P
                xt = work.tile([P, F], FP, tag="xs_stage")
                nc.sync.dma_start(out=xt[:rows, :],
                                  in_=x_local[t * P : t * P + rows, :])
                xtb = work.tile([P, F], BF, tag="xs_stage_b")
                nc.vector.tensor_scalar(out=xtb[:rows, :], in0=xt[:rows, :],
                                        scalar1=s_out[:rows, t : t + 1],
                                        scalar2=None, op0=mybir.AluOpType.mult)
                nc.sync.dma_start(out=ag1_in[t * P : t * P + rows, :],
                                  in_=xtb[:rows, :])

            nc.gpsimd.collective_compute(
                "AllGather", mybir.AluOpType.bypass,
                replica_groups=[list(range(NCORES))],
                ins=[ag1_in[:]], outs=[xs_full[:]],
            )

            # ---------- the two graph-conv layers ----------
            qrr = [0]

            def layer(xsrc, W_t, bb_t, is_last):
                for t in range(TPC):
                    rows = last_rows if t == TPC - 1 else P
                    C_t_total = int(C_th[t, 0] + C_th[t, 1])
                    # gather the tile's edges (one call per node-half)
                    gtiles = {}
                    for h in (0, 1):
                        C = int(C_th[t, h])
                        if C == 0:
                            continue
                        co = int(chunk_off[t, h])
                        g = gpool.tile([P, C, F], BF, tag=f"g{h}", name=f"g_{t}_{h}")
                        nidx = C * P
                        nc.gpsimd.dma_gather(
                            g[:],
                            xsrc[h * HALF : h * HALF + (n_nodes - HALF if h else HALF), :],
                            gidx_t[:, 8 * co : 8 * (co + C)],
                            nidx, nidx, F,
                            single_packet=False,
                            queue_num=qrr[0] % 4,
                        )
                        qrr[0] += 1
                        gtiles[h] = g

                    agg = pagg.tile([P, P], FP, tag="agg")
                    ci = 0
                    for h in (0, 1):
                        C = int(C_th[t, h])
                        if C == 0:
                            continue
                        co = int(chunk_off[t, h])
                        g = gtiles[h]
                        for c in range(C):
                            gc = co + c
                            sw = spool.tile([P, P], BF, tag="sw", name=f"sw_{t}_{h}_{c}")
                            nc.vector.tensor_scalar(
                                out=sw[:], in0=iota_t[:],
                                scalar1=dstrel_t[:, gc : gc + 1],
                                scalar2=wchunk_t[:, gc : gc + 1],
                                op0=mybir.AluOpType.is_equal,
                                op1=mybir.AluOpType.mult,
                            )
                            nc.tensor.matmul(
                                out=agg[:], lhsT=g[:, c, :], rhs=sw[:],
                                start=(ci == 0), stop=(ci == C_t_total - 1),
                            )
                            ci += 1

                    aggT_sb = work.tile([P, P], FP, tag="aggT")
                    nc.scalar.copy(aggT_sb[:], agg[:])
                    hp = ph.tile([P, F], FP, tag="hp")
                    nc.tensor.matmul(out=hp[:], lhsT=aggT_sb[:], rhs=W_t[:],
                                     start=True, stop=True)

                    v = work.tile([P, F], FP, tag="v")
                    nc.vector.tensor_scalar(out=v[:], in0=hp[:],
                                            scalar1=s_in[:, t : t + 1],
                                            scalar2=None,
                                            op0=mybir.AluOpType.mult)
                    nc.vector.tensor_tensor(out=v[:], in0=v[:], in1=bb_t[:],
                                            op=mybir.AluOpType.add)
                    u = work.tile([P, F], FP, tag="u")
                    nc.scalar.activation(u[:], v[:],
                                         mybir.ActivationFunctionType.Relu,
                                         scale=-1.0)
                    ex = work.tile([P, F], FP, tag="ex")
                    nc.scalar.activation(ex[:], u[:],
                                         mybir.ActivationFunctionType.Exp,
                                         scale=-1.0)
                    r = work.tile([P, F], FP, tag="r")
                    tt = work.tile([P, F], FP, tag="tt")
                    ot = work.tile([P, F], FP, tag="ot")
                    if is_last:
                        nc.scalar.activation(r[:], v[:],
                                             mybir.ActivationFunctionType.Relu,
                                             scale=LAM)
                        nc.vector.tensor_scalar(out=tt[:], in0=ex[:],
                                                scalar1=LAM * ALPHA,
                                                scalar2=-LAM * ALPHA,
                                                op0=mybir.AluOpType.mult,
                                                op1=mybir.AluOpType.add)
                        nc.vector.tensor_tensor(out=ot[:], in0=r[:], in1=tt[:],
                                                op=mybir.AluOpType.add)
                        nc.sync.dma_start(
                            out=out_ext[t * P : t * P + rows, :],
                            in_=ot[:rows, :])
                    else:
                        nc.scalar.activation(r[:], v[:],
                                             mybir.ActivationFunctionType.Relu,
                                             scale=s2[:, t : t + 1])
                        nc.vector.tensor_scalar(out=tt[:], in0=ex[:],
                                                scalar1=s3[:, t : t + 1],
                                                scalar2=s3n[:, t : t + 1],
                                                op0=mybir.AluOpType.mult,
                                                op1=mybir.AluOpType.add)
                        otb = work.tile([P, F], BF, tag="otb")
                        nc.vector.tensor_tensor(out=otb[:], in0=r[:], in1=tt[:],
                                                op=mybir.AluOpType.add)
                        nc.sync.dma_start(
                            out=ag2_in[t * P : t * P + rows, :],
                            in_=otb[:rows, :])

            layer(xs_full, W1_t, b1b_t, is_last=False)
            nc.gpsimd.collective_compute(
                "AllGather", mybir.AluOpType.bypass,
                replica_groups=[list(range(NCORES))],
                ins=[ag2_in[:]], outs=[xs2_full[:]],
            )
            layer(xs2_full, W2_t, b2b_t, is_last=True)

    nc.compile()
    return nc


_CACHE = {}


def _get_program(meta_key, meta):
    if meta_key not in _CACHE:
        _CACHE[meta_key] = _build_program(meta)
    return _CACHE[meta_key]


def kernel(x, src, dst, edge_w, W1, b1, W2, b2):
    x = np.asarray(x, dtype=np.float32)
    src_np = np.asarray(src)
    dst_np = np.asarray(dst)
    w_np = np.asarray(edge_w, dtype=np.float32)
    W1 = np.asarray(W1, dtype=np.float32)
    b1 = np.asarray(b1, dtype=np.float32)
    W2 = np.asarray(W2, dtype=np.float32)
    b2 = np.asarray(b2, dtype=np.float32)

    n_nodes = x.shape[0]
    meta, per_core = _preprocess(src_np, dst_np, w_np, n_nodes)
    NPC = meta["NPC"]

    meta_key = (
        n_nodes, src_np.shape[0],
        meta["CT"], meta["SIN"], meta["SOUT"],
        tuple(meta["C_th"].reshape(-1).tolist()),
        tuple(meta["md_in"].tolist()), tuple(meta["md_out"].tolist()),
    )
    nc = _get_program(meta_key, meta)

    iota = np.broadcast_to(np.arange(P, dtype=np.float32), (P, P)).astype(ml_dtypes.bfloat16)
    b1b = np.broadcast_to(b1, (P, F)).copy()
    b2b = np.broadcast_to(b2, (P, F)).copy()

    in_maps = []
    for k in range(NCORES):
        in_maps.append({
            "x_local": x[k * NPC : (k + 1) * NPC],
            "gidx": per_core["gidx"][k],
            "dstrel": per_core["dstrel"][k],
            "wchunk": per_core["wchunk"][k],
            "w_in_pad": per_core["w_in_pad"][k],
            "w_out_pad": per_core["w_out_pad"][k],
            "ideg": per_core["ideg"][k],
            "odeg": per_core["odeg"][k],
            "W1": W1, "W2": W2, "b1b": b1b, "b2b": b2b,
            "iota": iota,
        })

    res = run_bass_kernel_spmd(nc, in_maps, core_ids=list(range(NCORES)))
    out = np.concatenate([res.results[k]["out"] for k in range(NCORES)], axis=0)
    return out.astype(np.float32)
